# revision 11
# baseline (speedup 1.0000x reference)
"""Multi-head attention (B=4, N=1024, C=1024, H=16) on 8 TRN2 NeuronCores.

Sharding: core c handles batch b = c//2 and query-row half g = c%2.
Data parallel over B; within a batch pair, tensor parallel over heads for
the K/V projections: each core computes K^T and V (times the mask factor
e) for its 8 heads over all 1024 keys, the halves are exchanged with a
2-core AllGather through DRAM bounce buffers, and every core reads the
full 16-head K/V back in global head order.  Q is computed locally for
the core's own 512 query rows over all 16 heads.  Each core then runs
full attention + output projection for its 512 rows — output rows are
disjoint, so no all-reduce is needed after the projection.

Compute is bf16 on the TensorEngine with fp32 PSUM accumulation; paired
matmuls that share a stationary operand skip the second LDWEIGHTS via
the BIR ldweights=False flag (weights stay resident in the PE array).
Softmax is computed without max-subtraction (logits are bounded ~2.5 for
this problem) as exp(S^T)@[V*e, e] with V as the stationary operand; the
denominator lands as an extra PSUM row, its reciprocal is computed as
exp(-ln d) on ScalarE, broadcast across partitions on GpSimd, and
multiplied in on VectorE.  e = exp(-5*(1-mask)) folds the additive mask
penalty in exactly.
"""

import numpy as np
import ml_dtypes

import concourse.bass as bass
import concourse.mybir as mybir
import concourse.tile as tile
from concourse import bacc

N_CORES = 8
B, N, C = 4, 1024, 1024
H = 16
D = C // H  # 64
NQ = N // 2  # query rows per core: 512
P = 128
KC = C // P  # 8 contraction chunks
CH = C // 2  # feature half owned per core: 512
SCALE = D ** -0.5
PAIRS = [[0, 1], [2, 3], [4, 5], [6, 7]]

F32 = mybir.dt.float32
BF16 = mybir.dt.bfloat16
AF = mybir.ActivationFunctionType

# The kernel's ACT work is an Exp stream plus Ln+Exp pairs (softmax
# denominator reciprocal).  The default table-set chooser maps Exp to
# exp_and_others and Ln to natural_log, thrashing ACT_TABLE_LOADs (~2.7us
# each) between them.  Strip Exp/Ln from every set except the combined
# natural_log_exp_and_others (dict order — and with it act_func_set_id
# numbering — is preserved) so both functions resolve to one resident set.
_ORIG_GAT = bacc.get_activation_tables


def _gat_one_exp_ln_set(arch):
    tables = {k: set(v) for k, v in _ORIG_GAT(arch).items()}
    keep = "natural_log_exp_and_others"
    if keep in tables and AF.Exp in tables[keep] and AF.Ln in tables[keep]:
        for name, fns in tables.items():
            if name != keep:
                fns.discard(AF.Exp)
                fns.discard(AF.Ln)
    return tables


bacc.get_activation_tables = _gat_one_exp_ln_set


def build_nc():
    nc = bacc.Bacc(None, num_devices=N_CORES)

    xT = nc.declare_dram_parameter("xT", [C, N], BF16, isOutput=False)
    xqT = nc.declare_dram_parameter("xqT", [C, NQ], BF16, isOutput=False)
    W_q = nc.declare_dram_parameter("W_q", [C, C], BF16, isOutput=False)
    W_k = nc.declare_dram_parameter("W_k", [C, CH], BF16, isOutput=False)
    W_v = nc.declare_dram_parameter("W_v", [C, CH], BF16, isOutput=False)
    W_p = nc.declare_dram_parameter("W_p", [C, C], BF16, isOutput=False)
    e_in = nc.declare_dram_parameter("e", [N], F32, isOutput=False)
    b_in = nc.declare_dram_parameter("b", [C], F32, isOutput=False)
    out_ext = nc.declare_dram_parameter("out", [NQ, C], F32, isOutput=True)

    with tile.TileContext(nc) as tc, (
        tc.tile_pool(name="acts", bufs=1)
    ) as apool, (
        tc.tile_pool(name="early", bufs=1)
    ) as early, (
        tc.tile_pool(name="work", bufs=2)
    ) as work, (
        tc.tile_pool(name="chain", bufs=2)
    ) as chain, (
        tc.tile_pool(name="dram", bufs=1, space="DRAM")
    ) as dram, (
        tc.tile_pool(name="ps_big", bufs=2, space="PSUM")
    ) as ps_big, (
        tc.tile_pool(name="ps_pv", bufs=2, space="PSUM")
    ) as ps_pv:
        e_s = apool.tile([P, KC], F32)
        nc.sync.dma_start(e_s[:], e_in.rearrange("(o p) -> p o", p=P))

        # ---- PE warmup while the loads stream in --------------------
        warm = early.tile([P, 512], BF16, tag="warm")
        nc.vector.memset(warm[:], 0.0)
        wps = ps_big.tile([P, 1024], F32, tag="big")
        for i in range(10):
            nc.tensor.matmul(wps[:, 0:512], warm[:, 0:P], warm[:],
                             start=True, stop=True)

        # ---- static loads (chunked so matmuls can start early) ------
        Wk_s = early.tile([P, KC, CH], BF16)
        xT_s = early.tile([P, KC, N], BF16)
        Wv_s = early.tile([P, KC, CH], BF16)
        Wq_s = early.tile([P, KC, C], BF16)
        xqT_s = early.tile([P, KC, NQ], BF16)
        for kc in range(KC):
            nc.sync.dma_start(Wk_s[:, kc, :], W_k[kc * P:(kc + 1) * P, :])
            nc.sync.dma_start(xT_s[:, kc, :], xT[kc * P:(kc + 1) * P, :])
        for kc in range(KC):
            nc.sync.dma_start(Wv_s[:, kc, :], W_v[kc * P:(kc + 1) * P, :])
        for kc in range(KC):
            nc.sync.dma_start(xqT_s[:, kc, :], xqT[kc * P:(kc + 1) * P, :])
            nc.sync.dma_start(Wq_s[:, kc, :], W_q[kc * P:(kc + 1) * P, :])
        Wp_s = apool.tile([P, KC, C], BF16)
        nc.sync.dma_start(Wp_s[:], W_p.rearrange("(ko p) n -> p ko n", p=P))
        bias_s = apool.tile([P, C], F32)
        nc.sync.dma_start(bias_s[:], b_in[None, :].to_broadcast((P, C)))

        # DRAM bounce buffers for the pair K/V exchange.  AllGather
        # concatenates by replica rank, and rank g owns global heads
        # 8g..8g+7, so the gathered buffers are in global head order on
        # both cores.
        kin = dram.tile([CH, N], BF16)
        kout = dram.tile([C, N], BF16)
        vin = dram.tile([N, CH], BF16)
        vout = dram.tile([2, N, CH], BF16)

        KT_s = apool.tile([P, KC, N], BF16)
        QT_s = apool.tile([P, KC, NQ], BF16)
        V_s = apool.tile([P, KC, H, D + 1], BF16)
        PT_s = apool.tile([P, KC, NQ], BF16)
        for mc in range(KC):
            nc.vector.tensor_copy(
                V_s[:, mc, :, D:D + 1],
                e_s[:, mc:mc + 1, None].to_broadcast((P, H, 1)),
            )

        def k_chunk(j):
            # Own-half K^T chunk j (128 features x all 1024 keys).
            # kc-outer so both key-halves share each stationary W chunk;
            # the second matmul skips the LDWEIGHTS (weights resident).
            ps = ps_big.tile([P, 1024], F32, tag="big", name="ps_k")
            for kc in range(KC):
                for nh in range(2):
                    mm = nc.tensor.matmul(
                        ps[:, nh * 512:(nh + 1) * 512],
                        Wk_s[:, kc, j * P:(j + 1) * P],
                        xT_s[:, kc, nh * 512:(nh + 1) * 512],
                        start=(kc == 0),
                        stop=(kc == KC - 1),
                    )
                    if nh == 1:
                        mm.ins.ldweights = False
            kw = work.tile([P, N], BF16, tag="kw", name="kw")
            nc.vector.tensor_copy(kw[:], ps[:])
            nc.sync.dma_start(kin[j * P:(j + 1) * P, :], kw[:])

        def q_chunk(fc2):
            ps = ps_big.tile([P, 1024], F32, tag="big", name="ps_q")
            for half in range(2):
                fc = 2 * fc2 + half
                for kc in range(KC):
                    nc.tensor.matmul(
                        ps[:, half * 512:(half + 1) * 512],
                        Wq_s[:, kc, fc * P:(fc + 1) * P],
                        xqT_s[:, kc, :],
                        start=(kc == 0),
                        stop=(kc == KC - 1),
                    )
            nc.vector.tensor_copy(
                QT_s[:, 2 * fc2:2 * fc2 + 2, :],
                ps[:].rearrange("p (a b) -> p a b", a=2),
            )

        def v_chunk(mc):
            # Own-half V for key-chunk mc (128 keys x 8 heads), with the
            # mask factor e folded in before the exchange.
            psf = ps_big.tile([P, 1024], F32, tag="big", name="ps_v")
            ps = psf[:, 0:512]
            for kc in range(KC):
                nc.tensor.matmul(
                    ps[:],
                    xT_s[:, kc, mc * P:(mc + 1) * P],
                    Wv_s[:, kc, :],
                    start=(kc == 0),
                    stop=(kc == KC - 1),
                )
            vw = work.tile([P, CH], BF16, tag="vw", name="vw")
            nc.vector.tensor_scalar_mul(vw[:], ps[:], e_s[:, mc:mc + 1])
            nc.sync.dma_start(vin[mc * P:(mc + 1) * P, :], vw[:])

        def st_exp(hp, expT, pv_hp=None, pv_expT=None, pv=None):
            """S^T+exp tiles for pair hp.  When a previous pair's PV is
            supplied, its matmuls are emitted in two 8-matmul chunks
            between S^T groups: ScalarE keeps a 2-tile exp backlog to
            drain while TensorE runs the PV chunk, and the V stationary
            operand only swaps twice per round (LDWEIGHTS stays
            pipelined)."""
            for kcp in range(4):
                for h01 in range(2):
                    lo, hi = h01 * 64, h01 * 64 + 64
                    ps = ps_big.tile([P, 1024], F32, tag="big", name="ps_st")
                    for j in range(2):
                        kc = 2 * kcp + j
                        nc.tensor.matmul(
                            ps[:, j * 512:(j + 1) * 512],
                            KT_s[lo:hi, hp, kc * P:(kc + 1) * P],
                            QT_s[lo:hi, hp, :],
                            start=True,
                            stop=True,
                        )
                    nc.scalar.activation(
                        expT[h01][:, 2 * kcp:2 * kcp + 2, :],
                        ps[:].rearrange("p (a b) -> p a b", a=2),
                        AF.Exp,
                        scale=SCALE,
                    )
                if pv is not None and kcp in (1, 3):
                    for h01 in range(2):
                        for kc in range(4 * (kcp // 2), 4 * (kcp // 2) + 4):
                            nc.tensor.matmul(
                                pv[0:D + 1, h01, :],
                                V_s[:, kc, 2 * pv_hp + h01, :],
                                pv_expT[h01][:, kc, :],
                                start=(kc == 0),
                                stop=(kc == KC - 1),
                            )

        def pv_norm(hp, expT, pv=None):
            if pv is None:
                pv = ps_pv.tile([P, 2, NQ], F32, tag="pv")
                for h01 in range(2):
                    h = 2 * hp + h01
                    for kc in range(KC):
                        nc.tensor.matmul(
                            pv[0:D + 1, h01, :],
                            V_s[:, kc, h, :],
                            expT[h01][:, kc, :],
                            start=(kc == 0),
                            stop=(kc == KC - 1),
                        )
            lnr = chain.tile([1, 2, NQ], F32, tag="lnr")
            rcr = chain.tile([1, 2, NQ], F32, tag="rcr")
            nc.scalar.activation(lnr[0:1], pv[D:D + 1, :, :], AF.Ln)
            nc.scalar.activation(rcr[0:1], lnr[0:1], AF.Exp, scale=-1.0)
            for h01 in range(2):
                bcast = chain.tile([D, NQ], F32, tag=f"bcast{h01}",
                                   name=f"bcast{h01}")
                nc.gpsimd.partition_broadcast(bcast[:], rcr[0:1, h01, :])
                nc.vector.tensor_mul(
                    PT_s[h01 * D:(h01 + 1) * D, hp, :],
                    pv[0:D, h01, :],
                    bcast[:],
                )

        # ---- schedule ----------------------------------------------
        # Own K half -> AllGather; own V half -> AllGather; Q while the
        # gathers fly; then the S^T/exp/PV attention pipeline in global
        # head order (everything post-gather, so no head-order asymmetry
        # between the two cores of a pair).
        for j in range(4):
            k_chunk(j)
        nc.gpsimd.collective_compute(
            "AllGather",
            mybir.AluOpType.bypass,
            replica_groups=PAIRS,
            ins=[kin.opt()],
            outs=[kout.opt()],
        )
        nc.sync.dma_start(KT_s[:], kout.rearrange("(f p) n -> p f n", p=P))
        for mc in range(KC):
            v_chunk(mc)
        nc.gpsimd.collective_compute(
            "AllGather",
            mybir.AluOpType.bypass,
            replica_groups=PAIRS,
            ins=[vin.opt()],
            outs=[vout.opt()],
        )
        for s in range(2):
            for hh in range(8):
                nc.sync.dma_start(
                    V_s[:, :, s * 8 + hh, 0:D],
                    vout[s, :, hh * D:(hh + 1) * D].rearrange(
                        "(m p) d -> p m d", p=P),
                )
        for i in range(4):
            q_chunk(i)

        expTs = {}

        def new_expT(i):
            return [
                work.tile([P, KC, NQ], BF16, tag=f"exp{i % 2}_{h01}",
                          name=f"expT{h01}", bufs=1)
                for h01 in range(2)
            ]

        expTs[0] = new_expT(0)
        st_exp(0, expTs[0])
        for i in range(1, KC):
            expTs[i] = new_expT(i)
            pv = ps_pv.tile([P, 2, NQ], F32, tag="pv", name="pv")
            st_exp(i, expTs[i], i - 1, expTs[i - 1], pv)
            pv_norm(i - 1, expTs[i - 1], pv)
        pv_norm(KC - 1, expTs[KC - 1])

        # ---- output projection + bias -------------------------------
        for qs in range(4):
            ps = ps_big.tile([P, 1024], F32, tag="big", name="ps_pj")
            for fc in range(KC):
                for nn in range(2):
                    mm = nc.tensor.matmul(
                        ps[:, nn * 512:(nn + 1) * 512],
                        PT_s[:, fc, qs * P:(qs + 1) * P],
                        Wp_s[:, fc, nn * 512:(nn + 1) * 512],
                        start=(fc == 0),
                        stop=(fc == KC - 1),
                    )
                    if nn == 1:
                        mm.ins.ldweights = False
            o_sb = work.tile([P, 1024], F32, tag="osb")
            nc.vector.tensor_add(o_sb[:], ps[:], bias_s[:])
            nc.sync.dma_start(out_ext[qs * P:(qs + 1) * P, :], o_sb[:])

    nc.finalize()
    return nc


def make_in_maps(x, mask, W_qkv, W_proj, b_proj):
    bf = ml_dtypes.bfloat16
    x = np.asarray(x, np.float32)
    mask = np.asarray(mask, np.float32)
    W_qkv = np.asarray(W_qkv, np.float32)
    W_proj = np.asarray(W_proj, np.float32)
    b_proj = np.asarray(b_proj, np.float32)

    W_q = np.ascontiguousarray(W_qkv[:, 0:C]).astype(bf)
    W_p = np.ascontiguousarray(W_proj).astype(bf)
    e_all = np.exp(-5.0 * (1.0 - mask)).astype(np.float32)  # [B, N]

    in_maps = []
    for c in range(N_CORES):
        b, g = divmod(c, 2)
        xT = np.ascontiguousarray(x[b].T).astype(bf)
        xqT = np.ascontiguousarray(x[b, g * NQ:(g + 1) * NQ, :].T).astype(bf)
        W_k = np.ascontiguousarray(
            W_qkv[:, C + g * CH:C + (g + 1) * CH]).astype(bf)
        W_v = np.ascontiguousarray(
            W_qkv[:, 2 * C + g * CH:2 * C + (g + 1) * CH]).astype(bf)
        in_maps.append({
            "xT": xT, "xqT": xqT, "W_q": W_q, "W_k": W_k, "W_v": W_v,
            "W_p": W_p, "e": np.ascontiguousarray(e_all[b]),
            "b": b_proj,
        })
    return in_maps


def assemble_output(results):
    out = np.zeros((B, N, C), np.float32)
    for c in range(N_CORES):
        b, g = divmod(c, 2)
        out[b, g * NQ:(g + 1) * NQ, :] = results[c]["out"]
    return out


def kernel(x, mask, W_qkv, W_proj, b_proj):
    from concourse.bass_utils import run_bass_kernel_spmd

    nc = build_nc()
    in_maps = make_in_maps(x, mask, W_qkv, W_proj, b_proj)
    res = run_bass_kernel_spmd(nc, in_maps, core_ids=list(range(N_CORES)))
    return assemble_output(res.results)
